# revision 18
# baseline (speedup 1.0000x reference)
"""LoRA linear kernel for Trainium2 (8 NeuronCores, SPMD data-parallel).

Computes out = x @ (A @ B) for
    x: [4, 2048, 4096] f32, A: [4096, 16] f32, B: [16, 4096] f32
by reassociating to (x @ A) @ B  (2.1 GFLOP instead of 274 GFLOP).

Sharding: x is split row-wise (batch*seq = 8192 rows -> 1024 rows/core),
A and B replicated; no collectives. Everything runs in bf16 with f32
PSUM accumulation (error ~5e-3 of output scale, under the 2e-2 gate);
outputs are written bf16 and upcast on the host, halving write traffic.

Per 256-row chunk, two 128-row blocks ride the PE array concurrently
via tile_position (the rank is only 16, so an unpacked matmul uses
16/128 of the array):

  stage 1 (col strips): strip g accumulates tT_g[16, 128] = (x_g @ A).T
      into PSUM partitions 32g..32g+15 over KC=32 contraction chunks.
      (Column strips share the moving-operand XBUS, so this is mostly
      stream-serial; the strips' real job is placing tT_g in different
      partition groups for stage 2.)
  stage 2 (row strips): strip g computes out_g[128, d] = tT_g.T @ B_g
      with tT_g / a B replica at SBUF partitions 32g..32g+15,
      contracting in array row group 32g. Row strips execute
      concurrently (partition-disjoint slices of one XBUS), halving
      stage-2 time.

Scheduling structure (each engine has exactly one job, so no
instruction ever head-of-line blocks another kind of work):
  - ALL input DMAs are emitted before the chunk loop on the SP HWDGE
    ring; every chunk has its own resident SBUF buffer, so the input
    stream runs at line rate end to end.
  - ALL output DMAs ride the same SP ring after the inputs (program
    order), issued in half-osb pieces as soon as the copies land.
    (gpsimd SWDGE is deliberately NOT used: its queue drains add ~6 us
    to the Tile epilogue.)
  - PSUM->SBUF copies run as 1024-col pairs alternating scalar/vector
    (fp32-from-PSUM is capped at 1x mode on both engines).
  - A ~6 us warm-up matmul burst on a zeroed tile runs while the first
    input DMA is in flight: the PE's HAM clock gate defaults to 4/8
    (1.2 GHz) and only reaches 8/8 (2.4 GHz) after ~3.4 us of
    sustained activity, so warming during the DMA dead time roughly
    doubles matmul throughput once real data arrives.

Chunks are sized [128, 256, 256, 256, 128]: a small first chunk (its
DMA split along the contraction dim) starts the PE early, and a small
last chunk shortens the drain tail. The host pre-tiles x chunk-major
into xP[128, KC*RPC] (bf16) so every chunk DMA reads >=8 KB contiguous
per partition.
"""

import numpy as np
import ml_dtypes

import concourse.bass as bass
import concourse.bacc as bacc
import concourse.mybir as mybir
from concourse.tile import TileContext
from concourse.bass_utils import run_bass_kernel_spmd

N_CORES = 8
BATCH, SEQ, D_IN, D_OUT, R = 4, 2048, 4096, 4096, 16
ROWS = BATCH * SEQ              # 8192
RPC = ROWS // N_CORES           # 1024 rows per core
KC = D_IN // 128                # 32 contraction chunks of 128
CHUNKS = (128, 256, 256, 256, 128)   # rows per pipeline chunk
DCP = 1024                      # d_out columns per PSUM copy (2 banks)
NDCP = D_OUT // DCP             # 4
N_WARMUP = 14                   # warm-up matmuls (N=512 each, ~6 us cold)

F32 = mybir.dt.float32
BF16 = mybir.dt.bfloat16
NP_BF16 = ml_dtypes.bfloat16

_cache = {}


def _build(out_bf16=True):
    nc = bacc.Bacc("TRN2", target_bir_lowering=False)
    out_dt = BF16 if out_bf16 else F32

    # Chunk-major flat layout: per partition p, chunk k occupies a
    # contiguous [KC, rch_k] block (value = x_shard[n, c*128 + p]).
    xP = nc.dram_tensor("xP", [128, KC * RPC], BF16, kind="ExternalInput")
    Ap = nc.dram_tensor("Ap", [128, KC, R], BF16, kind="ExternalInput")
    Bw = nc.dram_tensor("Bw", [R, D_OUT], BF16, kind="ExternalInput")
    out = nc.dram_tensor("out", [RPC, D_OUT], out_dt, kind="ExternalOutput")

    with TileContext(nc) as tc:
        with (
            tc.tile_pool(name="consts", bufs=1) as cpool,
            tc.tile_pool(name="xin", bufs=1) as xpool,
            tc.tile_pool(name="tbuf", bufs=2) as tpool,
            tc.tile_pool(name="obuf", bufs=4) as opool,
            tc.tile_pool(name="pt", bufs=2, space="PSUM") as ptpool,
            tc.tile_pool(name="po", bufs=3, space="PSUM") as popool,
        ):
            a_tile = cpool.tile([128, KC, R], BF16)
            # B replicated into partition strips 32g..32g+15
            b2 = cpool.tile([128, D_OUT], BF16)

            # ---- all input DMAs, program-first on the SP ring ----
            xts = []
            off = 0
            for k, rch in enumerate(CHUNKS):
                xt = xpool.tile([128, KC * rch], BF16, name=f"xt{k}",
                                tag=f"xt{k}")
                xts.append(xt)
                if k == 0:
                    # A first (stage 1 needs it with the first x bytes),
                    # then the first chunk split along c so stage 1 starts
                    # after ~0.5 MB; B replicas follow (stage 2 needs them
                    # ~4 us later).
                    hl = (KC // 2) * rch
                    nc.sync.dma_start(out=a_tile[:], in_=Ap[:, :, :])
                    nc.sync.dma_start(out=xt[:, :hl],
                                      in_=xP[:, off:off + hl])
                    nc.sync.dma_start(out=xt[:, hl:],
                                      in_=xP[:, off + hl:off + KC * rch])
                    for g in range(2):
                        nc.sync.dma_start(out=b2[32 * g:32 * g + R, :],
                                          in_=Bw[:, :])
                else:
                    nc.sync.dma_start(out=xt[:],
                                      in_=xP[:, off:off + KC * rch])
                off += KC * rch

            # ---- PE warm-up during the input-DMA dead time ----
            wu = cpool.tile([128, 512], BF16)
            nc.vector.memset(wu[:], 0)
            wu_pt = ptpool.tile([R, 512], F32, name="wu_pt", tag="pt")
            for _ in range(N_WARMUP):
                nc.tensor.matmul(
                    wu_pt[:], wu[:, :R], wu[:, :],
                    start=True, stop=True, skip_group_check=True,
                )

            # ---- compute/copy/store per chunk ----
            row0 = 0
            for k, rch in enumerate(CHUNKS):
                nway = rch // 128
                xt = xts[k]

                # stage 1: strip g accumulates tT of row-block g into
                # PSUM partitions 32g..32g+15.
                pt = ptpool.tile([128, 128], F32, name="pt", tag="pt")
                for c in range(KC):
                    for g in range(nway):
                        nc.tensor.matmul(
                            pt[32 * g:32 * g + R, :],
                            a_tile[:, c, :],
                            xt[:, c * rch + 128 * g:c * rch + 128 * (g + 1)],
                            start=(c == 0),
                            stop=(c == KC - 1),
                            tile_position=(0, 32 * g),
                            skip_group_check=True,
                        )
                tT = tpool.tile([128, 128], BF16, name="tT", tag="tT")
                nc.scalar.copy(out=tT[:], in_=pt[:])

                # stage 2: strip g computes out rows of block g
                # (concurrent row strips); PSUM leaves in 1024-col pair
                # copies, scalar/vector alternating; each osb half is
                # DMA'd out as soon as its two pair copies land.
                osbs = [opool.tile([128, D_OUT], out_dt, name=f"osb{g}_{k}",
                                   tag="osb") for g in range(nway)]
                for dcp in range(NDCP):
                    pos = [popool.tile([128, DCP], F32, name=f"po{g}",
                                       tag="po") for g in range(nway)]
                    for half in range(2):
                        cols = slice(half * 512, (half + 1) * 512)
                        bcols = slice(dcp * DCP + half * 512,
                                      dcp * DCP + (half + 1) * 512)
                        for g in range(nway):
                            nc.tensor.matmul(
                                pos[g][:, cols],
                                tT[32 * g:32 * g + R, :],
                                b2[32 * g:32 * g + R, bcols],
                                start=True,
                                stop=True,
                                tile_position=(32 * g, 0),
                                skip_group_check=True,
                            )
                    for g in range(nway):
                        dst = osbs[g][:, dcp * DCP:(dcp + 1) * DCP]
                        if (g + dcp) % 2 == 0:
                            nc.scalar.copy(out=dst, in_=pos[g][:])
                        else:
                            nc.vector.tensor_copy(dst, pos[g][:])
                    if dcp % 2 == 1:
                        # stream out the finished half of each block's osb
                        hcols = slice((dcp - 1) * DCP, (dcp + 1) * DCP)
                        for g in range(nway):
                            r0 = row0 + 128 * g
                            nc.sync.dma_start(out=out[r0:r0 + 128, hcols],
                                              in_=osbs[g][:, hcols])
                row0 += rch
    nc.compile()
    return nc


def _get_nc(out_bf16=True):
    key = ("v8", out_bf16)
    if key not in _cache:
        _cache[key] = _build(out_bf16)
    return _cache[key]


def kernel(x, A, B, trace=False, out_bf16=True, **_ignored):
    x = np.asarray(x, dtype=np.float32)
    A = np.asarray(A, dtype=np.float32)
    B = np.asarray(B, dtype=np.float32)
    xf = x.reshape(ROWS, D_IN)

    Ab = np.ascontiguousarray(
        A.astype(NP_BF16).reshape(KC, 128, R).transpose(1, 0, 2))
    Bb = np.ascontiguousarray(B.astype(NP_BF16))

    nc = _get_nc(out_bf16)
    in_maps = []
    for i in range(N_CORES):
        xs = xf[i * RPC:(i + 1) * RPC].astype(NP_BF16)  # [1024, 4096]
        # chunk-major: per partition p, chunk k holds [KC, rch_k] with
        # xP[p, k][c, j] = xs[row0_k + j, c*128 + p]
        parts = []
        r0 = 0
        for rch in CHUNKS:
            blk = xs[r0:r0 + rch].reshape(rch, KC, 128).transpose(2, 1, 0)
            parts.append(blk.reshape(128, KC * rch))
            r0 += rch
        xPc = np.ascontiguousarray(np.concatenate(parts, axis=1))
        in_maps.append({"xP": xPc, "Ap": Ab, "Bw": Bb})

    res = run_bass_kernel_spmd(nc, in_maps, list(range(N_CORES)), trace=trace)
    outs = [res.results[i]["out"] for i in range(N_CORES)]
    full = np.concatenate(outs, axis=0).reshape(BATCH, SEQ, D_OUT)
    full = np.asarray(full, dtype=np.float32)
    if trace:
        kernel.last_exec_time_ns = res.exec_time_ns
        kernel.last_results = res
    return full


# revision 19
# speedup vs baseline: 1.1665x; 1.1665x over previous
"""LoRA linear kernel for Trainium2 (8 NeuronCores, SPMD data-parallel).

Computes out = x @ (A @ B) for
    x: [4, 2048, 4096] f32, A: [4096, 16] f32, B: [16, 4096] f32
by reassociating to (x @ A) @ B  (2.1 GFLOP instead of 274 GFLOP).

Sharding: x is split row-wise (batch*seq = 8192 rows -> 1024 rows/core),
A and B replicated; no collectives. Everything runs in bf16 with f32
PSUM accumulation (error ~5e-3 of output scale, under the 2e-2 gate);
outputs are written bf16 and upcast on the host, halving write traffic.

The kernel is co-limited by the serial PE matmul stream (~46 us at the
板's ~50% util clamp) and HBM DMA (16.8 MB at ~360 GB/s ~= 46 us),
which overlap almost fully; everything else (copies, issue, semaphores)
hides underneath. Per 256-row chunk, two 128-row blocks ride the PE
array concurrently via tile_position (rank is only 16, so an unpacked
matmul uses 16/128 of the array):

  stage 1 (col strips): strip g accumulates tT_g[16, 128] = (x_g @ A).T
      into PSUM partitions 32g..32g+15 over KC=32 contraction chunks.
      (Column strips share the moving-operand XBUS, so this is
      stream-serial; the strips' real job is placing tT_g in different
      partition groups for stage 2.)
  stage 2 (row strips): strip g computes out_g[128, d] = tT_g.T @ B_g
      with tT_g / a B replica at SBUF partitions 32g..32g+15,
      contracting in array row group 32g. Row strips execute
      concurrently (partition-disjoint slices of one XBUS), halving
      stage-2 time.

Scheduling:
  - ALL input DMAs are emitted before the chunk loop on the SP HWDGE
    ring (per-chunk resident buffers), so the input stream runs at
    line rate with no issue stalls.
  - Output DMAs alternate between the ACT and SP HWDGE rings, one full
    osb per 128-row block; the last chunk's output is split in half
    across both rings to shorten the drain tail. (gpsimd SWDGE is
    deliberately NOT used: its queue drains add ~6 us to the epilogue.)
  - PSUM->SBUF copies run as 1024-col pairs alternating scalar/vector
    (fp32-from-PSUM is capped at 1x mode on both engines).

Chunks are sized [128, 256, 256, 256, 128]: a small first chunk (its
DMA split along the contraction dim) starts the PE early, and a small
last chunk shortens the drain tail. The host pre-tiles x chunk-major
into xP[128, KC*RPC] (bf16) so every chunk DMA reads >=8 KB contiguous
per partition (large descriptors -> near-peak HBM bandwidth).
"""

import numpy as np
import ml_dtypes

import concourse.bass as bass
import concourse.bacc as bacc
import concourse.mybir as mybir
from concourse.tile import TileContext
from concourse.bass_utils import run_bass_kernel_spmd

N_CORES = 8
BATCH, SEQ, D_IN, D_OUT, R = 4, 2048, 4096, 4096, 16
ROWS = BATCH * SEQ              # 8192
RPC = ROWS // N_CORES           # 1024 rows per core
KC = D_IN // 128                # 32 contraction chunks of 128
CHUNKS = (128, 256, 256, 256, 128)   # rows per pipeline chunk
DCP = 1024                      # d_out columns per PSUM copy (2 banks)
NDCP = D_OUT // DCP             # 4

F32 = mybir.dt.float32
BF16 = mybir.dt.bfloat16
NP_BF16 = ml_dtypes.bfloat16

_cache = {}


def _build(out_bf16=True):
    nc = bacc.Bacc("TRN2", target_bir_lowering=False)
    out_dt = BF16 if out_bf16 else F32

    # Chunk-major flat layout: per partition p, chunk k occupies a
    # contiguous [KC, rch_k] block (value = x_shard[n, c*128 + p]).
    xP = nc.dram_tensor("xP", [128, KC * RPC], BF16, kind="ExternalInput")
    Ap = nc.dram_tensor("Ap", [128, KC, R], BF16, kind="ExternalInput")
    Bw = nc.dram_tensor("Bw", [R, D_OUT], BF16, kind="ExternalInput")
    out = nc.dram_tensor("out", [RPC, D_OUT], out_dt, kind="ExternalOutput")

    with TileContext(nc) as tc:
        with (
            tc.tile_pool(name="consts", bufs=1) as cpool,
            tc.tile_pool(name="xin", bufs=1) as xpool,
            tc.tile_pool(name="tbuf", bufs=2) as tpool,
            tc.tile_pool(name="obuf", bufs=4) as opool,
            tc.tile_pool(name="pt", bufs=2, space="PSUM") as ptpool,
            tc.tile_pool(name="po", bufs=3, space="PSUM") as popool,
        ):
            a_tile = cpool.tile([128, KC, R], BF16)
            # B replicated into partition strips 32g..32g+15
            b2 = cpool.tile([128, D_OUT], BF16)

            # ---- all input DMAs, program-first on the SP ring ----
            xts = []
            off = 0
            for k, rch in enumerate(CHUNKS):
                xt = xpool.tile([128, KC * rch], BF16, name=f"xt{k}",
                                tag=f"xt{k}")
                xts.append(xt)
                if k == 0:
                    # A first (stage 1 needs it with the first x bytes),
                    # then the first chunk split along c so stage 1 starts
                    # after ~0.5 MB; B replicas follow (stage 2 needs them
                    # ~4 us later).
                    hl = (KC // 2) * rch
                    nc.sync.dma_start(out=a_tile[:], in_=Ap[:, :, :])
                    nc.sync.dma_start(out=xt[:, :hl],
                                      in_=xP[:, off:off + hl])
                    nc.sync.dma_start(out=xt[:, hl:],
                                      in_=xP[:, off + hl:off + KC * rch])
                    for g in range(2):
                        nc.sync.dma_start(out=b2[32 * g:32 * g + R, :],
                                          in_=Bw[:, :])
                else:
                    nc.sync.dma_start(out=xt[:],
                                      in_=xP[:, off:off + KC * rch])
                off += KC * rch

            # ---- compute/copy/store per chunk ----
            row0 = 0
            out_ring = [nc.scalar, nc.sync]
            n_out = 0
            last_k = len(CHUNKS) - 1
            for k, rch in enumerate(CHUNKS):
                nway = rch // 128
                xt = xts[k]

                # stage 1: strip g accumulates tT of row-block g into
                # PSUM partitions 32g..32g+15.
                pt = ptpool.tile([128, 128], F32, name="pt", tag="pt")
                for c in range(KC):
                    for g in range(nway):
                        nc.tensor.matmul(
                            pt[32 * g:32 * g + R, :],
                            a_tile[:, c, :],
                            xt[:, c * rch + 128 * g:c * rch + 128 * (g + 1)],
                            start=(c == 0),
                            stop=(c == KC - 1),
                            tile_position=(0, 32 * g),
                            skip_group_check=True,
                        )
                tT = tpool.tile([128, 128], BF16, name="tT", tag="tT")
                nc.scalar.copy(out=tT[:], in_=pt[:])

                # stage 2: strip g computes out rows of block g
                # (concurrent row strips); PSUM leaves in 1024-col pair
                # copies, scalar/vector alternating.
                osbs = [opool.tile([128, D_OUT], out_dt, name=f"osb{g}_{k}",
                                   tag="osb") for g in range(nway)]
                for dcp in range(NDCP):
                    pos = [popool.tile([128, DCP], F32, name=f"po{g}",
                                       tag="po") for g in range(nway)]
                    for half in range(2):
                        cols = slice(half * 512, (half + 1) * 512)
                        bcols = slice(dcp * DCP + half * 512,
                                      dcp * DCP + (half + 1) * 512)
                        for g in range(nway):
                            nc.tensor.matmul(
                                pos[g][:, cols],
                                tT[32 * g:32 * g + R, :],
                                b2[32 * g:32 * g + R, bcols],
                                start=True,
                                stop=True,
                                tile_position=(32 * g, 0),
                                skip_group_check=True,
                            )
                    for g in range(nway):
                        dst = osbs[g][:, dcp * DCP:(dcp + 1) * DCP]
                        if (g + dcp) % 2 == 0:
                            nc.scalar.copy(out=dst, in_=pos[g][:])
                        else:
                            nc.vector.tensor_copy(dst, pos[g][:])
                for g in range(nway):
                    r0 = row0 + 128 * g
                    if k == last_k:
                        # split the final output across both HWDGE rings
                        # so the tail drains ~2x faster
                        half_d = D_OUT // 2
                        nc.scalar.dma_start(out=out[r0:r0 + 128, :half_d],
                                            in_=osbs[g][:, :half_d])
                        nc.sync.dma_start(out=out[r0:r0 + 128, half_d:],
                                          in_=osbs[g][:, half_d:])
                    else:
                        out_ring[n_out % 2].dma_start(
                            out=out[r0:r0 + 128, :], in_=osbs[g][:])
                        n_out += 1
                row0 += rch
    nc.compile()
    return nc


def _get_nc(out_bf16=True):
    key = ("v9", out_bf16)
    if key not in _cache:
        _cache[key] = _build(out_bf16)
    return _cache[key]


def kernel(x, A, B, trace=False, out_bf16=True, **_ignored):
    x = np.asarray(x, dtype=np.float32)
    A = np.asarray(A, dtype=np.float32)
    B = np.asarray(B, dtype=np.float32)
    xf = x.reshape(ROWS, D_IN)

    Ab = np.ascontiguousarray(
        A.astype(NP_BF16).reshape(KC, 128, R).transpose(1, 0, 2))
    Bb = np.ascontiguousarray(B.astype(NP_BF16))

    nc = _get_nc(out_bf16)
    in_maps = []
    for i in range(N_CORES):
        xs = xf[i * RPC:(i + 1) * RPC].astype(NP_BF16)  # [1024, 4096]
        # chunk-major: per partition p, chunk k holds [KC, rch_k] with
        # xP[p, k][c, j] = xs[row0_k + j, c*128 + p]
        parts = []
        r0 = 0
        for rch in CHUNKS:
            blk = xs[r0:r0 + rch].reshape(rch, KC, 128).transpose(2, 1, 0)
            parts.append(blk.reshape(128, KC * rch))
            r0 += rch
        xPc = np.ascontiguousarray(np.concatenate(parts, axis=1))
        in_maps.append({"xP": xPc, "Ap": Ab, "Bw": Bb})

    res = run_bass_kernel_spmd(nc, in_maps, list(range(N_CORES)), trace=trace)
    outs = [res.results[i]["out"] for i in range(N_CORES)]
    full = np.concatenate(outs, axis=0).reshape(BATCH, SEQ, D_OUT)
    full = np.asarray(full, dtype=np.float32)
    if trace:
        kernel.last_exec_time_ns = res.exec_time_ns
        kernel.last_results = res
    return full
